# revision 2
# baseline (speedup 1.0000x reference)
"""GNN message-passing (4x SpMM + drug-row squared norms) on 8 trn2 NeuronCores.

Design:
- Nodes are permuted into 784 windows of 128 rows (load-balanced by degree);
  core c owns windows [c*98, (c+1)*98) = 12544 row slots.
- d-state table lives in HBM as [100352, 128] fp16 (64 real dims + 64 pad so
  each row is a 256B dma_gather element).
- Per window: 4 dma_gathers (edge cols bucketed into 4 int16-addressable
  32768-slot groups), one-hot built via DVE tensor_scalar(iota == rowlocal)
  * val, PE matmul-accumulates scatter into PSUM [128 rows, 64].
- After each of steps 1..3, slabs are AllGathered into the shared table.
- acc (= e0+d1+d2+d3+d4) kept in SBUF fp32; final: dma_gather of owned drug
  rows from acc in DRAM, square + reduce on DVE.
Host does sharding/permutation prep and final gamma assembly (gamma/25).
"""
import numpy as np

N_NODES = 100000
N_EDGES = 3200000
DIM = 64
N_DRUGS = 8192
NCORES = 8
NW = 784            # total windows
WR = 128            # rows per window
WPC = NW // NCORES  # 98 windows per core
SLOTS = NW * WR     # 100352
RPC = WPC * WR      # 12544 rows per core
NGRP = 4
GSIZE = 32768
DPAD = 1280         # padded drugs per core
NSTEPS = 4


def _prep(emb, edge_vals, edge_row, edge_col, drugs):
    deg = np.bincount(edge_row, minlength=N_NODES)
    order = np.argsort(-deg, kind="stable")
    slot = np.empty(N_NODES, np.int64)
    ar = np.arange(N_NODES)
    slot[order] = (ar % NW) * WR + (ar // NW)

    er = slot[edge_row.astype(np.int64)]
    w = er >> 7
    rloc = (er & 127).astype(np.float32)
    cs = slot[edge_col.astype(np.int64)]
    g = cs >> 15
    gi = (cs & 32767).astype(np.int16)

    key = w * NGRP + g
    eord = np.argsort(key, kind="stable")
    key_s = key[eord]
    cnt = np.bincount(key_s, minlength=NW * NGRP).reshape(NW, NGRP)
    Cg = np.maximum(np.ceil(cnt.max(axis=0) / 128).astype(np.int64), 1)
    C_TOT = int(Cg.sum())
    off_g = np.zeros(NGRP, np.int64)
    off_g[1:] = np.cumsum(Cg)[:-1]
    SW = C_TOT * 8

    seg_start = np.zeros(NW * NGRP, np.int64)
    seg_start[1:] = np.cumsum(cnt.reshape(-1))[:-1]
    rank = np.arange(N_EDGES) - seg_start[key_s]
    ws = key_s // NGRP
    gs = key_s % NGRP

    rowloc_a = np.zeros((128, NW * C_TOT), np.float32)
    vals_a = np.zeros((128, NW * C_TOT), np.float32)
    ccol = ws * C_TOT + off_g[gs] + rank // 128
    cpart = rank % 128
    rowloc_a[cpart, ccol] = rloc[eord]
    vals_a[cpart, ccol] = edge_vals[eord].astype(np.float32)

    idx16 = np.zeros((16, NW * SW), np.int16)
    icol = ws * SW + off_g[gs] * 8 + rank // 16
    ipart = rank % 16
    idx16[ipart, icol] = gi[eord]
    idx_full = np.tile(idx16, (8, 1))

    emb16 = np.zeros((SLOTS, 128), np.float16)
    emb16[slot, :DIM] = emb.astype(np.float16)

    iota = np.broadcast_to(np.arange(128, dtype=np.float16), (128, 128)).copy()

    dslot = slot[drugs.astype(np.int64)]
    dcore = dslot // RPC
    dloc = (dslot % RPC).astype(np.int16)
    drug_idx = np.zeros((NCORES, 16, DPAD // 16), np.int16)
    drug_pos = []  # per core: original positions, in device token order
    for c in range(NCORES):
        pos = np.nonzero(dcore == c)[0]
        assert len(pos) <= DPAD, f"core {c} owns {len(pos)} drugs > {DPAD}"
        drug_pos.append(pos)
        ii = np.arange(len(pos))
        drug_idx[c, ii % 16, ii // 16] = dloc[pos]
    drug_idx_full = np.tile(drug_idx, (1, 8, 1))

    in_maps = []
    for c in range(NCORES):
        in_maps.append({
            "emb_slab": emb16[c * RPC:(c + 1) * RPC],
            "idx16": np.ascontiguousarray(
                idx_full[:, c * WPC * SW:(c + 1) * WPC * SW]),
            "rowloc": np.ascontiguousarray(
                rowloc_a[:, c * WPC * C_TOT:(c + 1) * WPC * C_TOT]),
            "vals": np.ascontiguousarray(
                vals_a[:, c * WPC * C_TOT:(c + 1) * WPC * C_TOT]),
            "iota": iota,
            "drugidx": drug_idx_full[c],
        })
    return in_maps, drug_pos, Cg, C_TOT


def _build(Cg, C_TOT):
    import concourse.bass as bass
    import concourse.mybir as mybir
    import concourse.tile as tile
    import concourse.bacc as bacc

    SW = C_TOT * 8
    off_g = np.zeros(NGRP, np.int64)
    off_g[1:] = np.cumsum(Cg)[:-1]

    nc = bacc.Bacc("TRN2", target_bir_lowering=False, debug=False,
                   num_devices=NCORES)
    fp16 = mybir.dt.float16
    f32 = mybir.dt.float32
    i16 = mybir.dt.int16

    t_emb = nc.dram_tensor("emb_slab", [RPC, 128], fp16, kind="ExternalInput")
    t_idx = nc.dram_tensor("idx16", [128, WPC * SW], i16, kind="ExternalInput")
    t_rl = nc.dram_tensor("rowloc", [128, WPC * C_TOT], f32, kind="ExternalInput")
    t_vl = nc.dram_tensor("vals", [128, WPC * C_TOT], f32, kind="ExternalInput")
    t_io = nc.dram_tensor("iota", [128, 128], fp16, kind="ExternalInput")
    t_di = nc.dram_tensor("drugidx", [128, DPAD // 16], i16, kind="ExternalInput")
    t_out = nc.dram_tensor("gamma", [128, DPAD // 128], f32, kind="ExternalOutput")

    with tile.TileContext(nc) as tc:
        with (
            tc.tile_pool(name="sb", bufs=1) as sb,
            tc.tile_pool(name="xgp", bufs=2) as xgp,
            tc.tile_pool(name="ohp", bufs=4) as ohp,
            tc.tile_pool(name="osp", bufs=3) as osp,
            tc.tile_pool(name="psp", bufs=2, space="PSUM") as psp,
            tc.tile_pool(name="drp", bufs=1, space="DRAM") as drp,
        ):
            idx_t = sb.tile([128, WPC * SW], i16)
            rl_t = sb.tile([128, WPC * C_TOT], f32)
            vl_t = sb.tile([128, WPC * C_TOT], f32)
            io_t = sb.tile([128, 128], fp16)
            di_t = sb.tile([128, DPAD // 16], i16)
            acc_t = sb.tile([128, WPC * DIM], f32)

            nc.sync.dma_start(out=idx_t[:], in_=t_idx[:, :])
            nc.sync.dma_start(out=rl_t[:], in_=t_rl[:, :])
            nc.sync.dma_start(out=vl_t[:], in_=t_vl[:, :])
            nc.sync.dma_start(out=io_t[:], in_=t_io[:, :])
            nc.sync.dma_start(out=di_t[:], in_=t_di[:, :])
            # merge setup DMA deps onto the DVE engine clock
            touch = sb.tile([128, 8], f32)
            nc.vector.tensor_copy(out=touch[:, 0:1], in_=rl_t[:, 0:1])
            nc.vector.tensor_copy(out=touch[:, 1:2], in_=vl_t[:, 0:1])
            nc.vector.tensor_copy(out=touch[:, 2:3], in_=io_t[:, 0:1])
            nc.vector.tensor_copy(out=touch[:, 3:4], in_=idx_t[:, 0:1])
            nc.vector.tensor_copy(out=touch[:, 4:5], in_=di_t[:, 0:1])

            bounce = drp.tile([RPC, 128], fp16)
            tables = [
                drp.tile([SLOTS, 128], fp16, addr_space="Shared",
                         name=f"tbl{k}")
                for k in range(NSTEPS)
            ]
            acc_d = drp.tile([RPC, DIM], f32)

            # acc := e0 slab (fp16 -> fp32 cast during DMA, SWDGE)
            nc.gpsimd.dma_start(
                out=acc_t[:],
                in_=bass.AP(t_emb, 0, [[128, 128], [WR * 128, WPC], [1, DIM]]),
            )
            # initial all-gather of e0 slabs into the shared table
            nc.sync.dma_start(out=bounce[:, :], in_=t_emb[:, :])
            nc.gpsimd.collective_compute(
                "AllGather", mybir.AluOpType.bypass,
                replica_groups=[list(range(NCORES))],
                ins=[bounce[:, :].opt()], outs=[tables[0][:, :].opt()],
            )

            gsz = [GSIZE, GSIZE, GSIZE, SLOTS - 3 * GSIZE]

            for step in range(NSTEPS):
                def body(iv, step=step):
                    xg_t = xgp.tile([128, C_TOT, 128], fp16, name="xg")
                    for g in range(NGRP):
                        nt = int(Cg[g]) * 128
                        nc.gpsimd.dma_gather(
                            out_ap=xg_t[:, int(off_g[g]):int(off_g[g] + Cg[g]), :],
                            in_ap=tables[step][int(g * GSIZE):int(g * GSIZE + gsz[g]), :],
                            idxs_ap=idx_t[:, bass.ds(iv * SW + int(off_g[g] * 8),
                                                     int(Cg[g]) * 8)],
                            num_idxs=nt, num_idxs_reg=nt,
                            elem_size=128, elem_step=128,
                            single_packet=False,
                        )
                    ps_t = psp.tile([128, DIM], f32, space="PSUM", name="ps")
                    for cj in range(C_TOT):
                        oh_t = ohp.tile([128, 128], fp16, name="oh")
                        nc.vector.tensor_scalar(
                            out=oh_t[:], in0=io_t[:],
                            scalar1=rl_t[:, bass.ds(iv * C_TOT + cj, 1)],
                            scalar2=vl_t[:, bass.ds(iv * C_TOT + cj, 1)],
                            op0=mybir.AluOpType.is_equal,
                            op1=mybir.AluOpType.mult,
                        )
                        nc.tensor.matmul(
                            ps_t[:], lhsT=oh_t[:], rhs=xg_t[:, cj, 0:DIM],
                            start=(cj == 0), stop=(cj == C_TOT - 1),
                        )
                    # acc += d_step
                    nc.vector.tensor_tensor(
                        out=acc_t[:, bass.ds(iv * DIM, DIM)],
                        in0=acc_t[:, bass.ds(iv * DIM, DIM)],
                        in1=ps_t[:], op=mybir.AluOpType.add,
                    )
                    if step < NSTEPS - 1:
                        os_t = osp.tile([128, 128], fp16, name="os")
                        nc.scalar.activation(
                            out=os_t[:, 0:DIM], in_=ps_t[:],
                            func=mybir.ActivationFunctionType.Copy)
                        nc.vector.memset(os_t[:, DIM:128], 0.0)
                        nc.sync.dma_start(
                            out=bounce[bass.ts(iv, WR), :], in_=os_t[:, :])

                with tc.For_i(0, WPC, 1) as iv:
                    body(iv)

                if step < NSTEPS - 1:
                    nc.gpsimd.collective_compute(
                        "AllGather", mybir.AluOpType.bypass,
                        replica_groups=[list(range(NCORES))],
                        ins=[bounce[:, :].opt()],
                        outs=[tables[step + 1][:, :].opt()],
                    )

            # final: gamma for owned drug rows
            nc.sync.dma_start(
                out=bass.AP(acc_d.tensor, 0,
                            [[DIM, 128], [WR * DIM, WPC], [1, DIM]]),
                in_=acc_t[:],
            )
            dg_t = sb.tile([128, DPAD // 128, DIM], f32)
            nc.gpsimd.dma_gather(
                out_ap=dg_t[:, :, :], in_ap=acc_d[:, :], idxs_ap=di_t[:, :],
                num_idxs=DPAD, num_idxs_reg=DPAD,
                elem_size=DIM, elem_step=DIM, single_packet=False,
            )
            sq_t = sb.tile([128, DPAD // 128, DIM], f32)
            nc.vector.tensor_tensor(
                out=sq_t[:, :, :], in0=dg_t[:, :, :], in1=dg_t[:, :, :],
                op=mybir.AluOpType.mult)
            gm_t = sb.tile([128, DPAD // 128, 1], f32)
            nc.vector.tensor_reduce(
                out=gm_t[:, :, :], in_=sq_t[:, :, :],
                axis=mybir.AxisListType.X, op=mybir.AluOpType.add)
            nc.sync.dma_start(out=t_out[:, :], in_=gm_t[:, :, 0])

    nc.compile()
    return nc


def kernel(emb, edge_vals, edge_row, edge_col, drugs):
    from concourse.bass_utils import run_bass_kernel_spmd

    in_maps, drug_pos, Cg, C_TOT = _prep(emb, edge_vals, edge_row, edge_col,
                                         drugs)
    nc = _build(Cg, C_TOT)
    res = run_bass_kernel_spmd(nc, in_maps, core_ids=list(range(NCORES)))
    gamma = np.zeros(N_DRUGS, np.float32)
    for c in range(NCORES):
        out = res.results[c]["gamma"]  # [128, DPAD//128]
        pos = drug_pos[c]
        ii = np.arange(len(pos))
        gamma[pos] = out[ii % 128, ii // 128] / 25.0
    return gamma


# revision 3
# speedup vs baseline: 1.4546x; 1.4546x over previous
"""GNN message-passing (4x SpMM + drug-row squared norms) on 8 trn2 NeuronCores.

Design:
- Nodes are permuted into 784 windows of 128 rows (load-balanced by degree);
  core c owns windows [c*98, (c+1)*98) = 12544 row slots.
- d-state table lives in HBM as [100352, 128] fp16 (64 real dims + 64 pad so
  each row is a 256B dma_gather element).
- Per window: 4 dma_gathers (edge cols bucketed into 4 int16-addressable
  32768-slot groups), one-hot built via DVE tensor_scalar(iota == rowlocal)
  * val, PE matmul-accumulates scatter into PSUM [128 rows, 64].
- After each of steps 1..3, slabs are AllGathered into the shared table.
- acc (= e0+d1+d2+d3+d4) kept in SBUF fp32; final: dma_gather of owned drug
  rows from acc in DRAM, square + reduce on DVE.
Host does sharding/permutation prep and final gamma assembly (gamma/25).
"""
import numpy as np

N_NODES = 100000
N_EDGES = 3200000
DIM = 64
N_DRUGS = 8192
NCORES = 8
NW = 784            # total windows
WR = 128            # rows per window
WPC = NW // NCORES  # 98 windows per core
SLOTS = NW * WR     # 100352
RPC = WPC * WR      # 12544 rows per core
NGRP = 4
GSIZE = 32768
DPAD = 1280         # padded drugs per core
NSTEPS = 4


def _prep(emb, edge_vals, edge_row, edge_col, drugs):
    deg = np.bincount(edge_row, minlength=N_NODES)
    order = np.argsort(-deg, kind="stable")
    slot = np.empty(N_NODES, np.int64)
    ar = np.arange(N_NODES)
    slot[order] = (ar % NW) * WR + (ar // NW)

    er = slot[edge_row.astype(np.int64)]
    w = er >> 7
    rloc = (er & 127).astype(np.float32)
    cs = slot[edge_col.astype(np.int64)]
    g = cs >> 15
    gi = (cs & 32767).astype(np.int16)

    key = w * NGRP + g
    eord = np.argsort(key, kind="stable")
    key_s = key[eord]
    cnt = np.bincount(key_s, minlength=NW * NGRP).reshape(NW, NGRP)
    Cg = np.maximum(np.ceil(cnt.max(axis=0) / 128).astype(np.int64), 1)
    C_TOT = int(Cg.sum())
    off_g = np.zeros(NGRP, np.int64)
    off_g[1:] = np.cumsum(Cg)[:-1]
    SW = C_TOT * 8

    seg_start = np.zeros(NW * NGRP, np.int64)
    seg_start[1:] = np.cumsum(cnt.reshape(-1))[:-1]
    rank = np.arange(N_EDGES) - seg_start[key_s]
    ws = key_s // NGRP
    gs = key_s % NGRP

    rowloc_a = np.zeros((128, NW * C_TOT), np.float32)
    vals_a = np.zeros((128, NW * C_TOT), np.float32)
    ccol = ws * C_TOT + off_g[gs] + rank // 128
    cpart = rank % 128
    rowloc_a[cpart, ccol] = rloc[eord]
    vals_a[cpart, ccol] = edge_vals[eord].astype(np.float32)

    idx16 = np.zeros((16, NW * SW), np.int16)
    icol = ws * SW + off_g[gs] * 8 + rank // 16
    ipart = rank % 16
    idx16[ipart, icol] = gi[eord]
    idx_full = np.tile(idx16, (8, 1))

    emb16 = np.zeros((SLOTS, 128), np.float16)
    emb16[slot, :DIM] = emb.astype(np.float16)

    iota = np.broadcast_to(np.arange(128, dtype=np.float16), (128, 128)).copy()

    dslot = slot[drugs.astype(np.int64)]
    dcore = dslot // RPC
    dloc = (dslot % RPC).astype(np.int16)
    drug_idx = np.zeros((NCORES, 16, DPAD // 16), np.int16)
    drug_pos = []  # per core: original positions, in device token order
    for c in range(NCORES):
        pos = np.nonzero(dcore == c)[0]
        assert len(pos) <= DPAD, f"core {c} owns {len(pos)} drugs > {DPAD}"
        drug_pos.append(pos)
        ii = np.arange(len(pos))
        drug_idx[c, ii % 16, ii // 16] = dloc[pos]
    drug_idx_full = np.tile(drug_idx, (1, 8, 1))

    in_maps = []
    for c in range(NCORES):
        in_maps.append({
            "emb_slab": emb16[c * RPC:(c + 1) * RPC],
            "idx16": np.ascontiguousarray(
                idx_full[:, c * WPC * SW:(c + 1) * WPC * SW]),
            "rowloc": np.ascontiguousarray(
                rowloc_a[:, c * WPC * C_TOT:(c + 1) * WPC * C_TOT]),
            "vals": np.ascontiguousarray(
                vals_a[:, c * WPC * C_TOT:(c + 1) * WPC * C_TOT]),
            "iota": iota,
            "drugidx": drug_idx_full[c],
        })
    return in_maps, drug_pos, Cg, C_TOT


def _build(Cg, C_TOT):
    import concourse.bass as bass
    import concourse.mybir as mybir
    import concourse.tile as tile
    import concourse.bacc as bacc

    SW = C_TOT * 8
    off_g = np.zeros(NGRP, np.int64)
    off_g[1:] = np.cumsum(Cg)[:-1]

    nc = bacc.Bacc("TRN2", target_bir_lowering=False, debug=False,
                   num_devices=NCORES, num_swdge_queues=4)
    fp16 = mybir.dt.float16
    f32 = mybir.dt.float32
    i16 = mybir.dt.int16

    t_emb = nc.dram_tensor("emb_slab", [RPC, 128], fp16, kind="ExternalInput")
    t_idx = nc.dram_tensor("idx16", [128, WPC * SW], i16, kind="ExternalInput")
    t_rl = nc.dram_tensor("rowloc", [128, WPC * C_TOT], f32, kind="ExternalInput")
    t_vl = nc.dram_tensor("vals", [128, WPC * C_TOT], f32, kind="ExternalInput")
    t_io = nc.dram_tensor("iota", [128, 128], fp16, kind="ExternalInput")
    t_di = nc.dram_tensor("drugidx", [128, DPAD // 16], i16, kind="ExternalInput")
    t_out = nc.dram_tensor("gamma", [128, DPAD // 128], f32, kind="ExternalOutput")

    with tile.TileContext(nc) as tc:
        with (
            tc.tile_pool(name="sb", bufs=1) as sb,
            tc.tile_pool(name="xgp", bufs=2) as xgp,
            tc.tile_pool(name="ohp", bufs=4) as ohp,
            tc.tile_pool(name="osp", bufs=3) as osp,
            tc.tile_pool(name="psp", bufs=2, space="PSUM") as psp,
            tc.tile_pool(name="drp", bufs=1, space="DRAM") as drp,
        ):
            idx_t = sb.tile([128, WPC * SW], i16)
            rl_t = sb.tile([128, WPC * C_TOT], f32)
            vl_t = sb.tile([128, WPC * C_TOT], f32)
            io_t = sb.tile([128, 128], fp16)
            di_t = sb.tile([128, DPAD // 16], i16)
            acc_t = sb.tile([128, WPC * DIM], f32)

            nc.sync.dma_start(out=idx_t[:], in_=t_idx[:, :])
            nc.sync.dma_start(out=rl_t[:], in_=t_rl[:, :])
            nc.sync.dma_start(out=vl_t[:], in_=t_vl[:, :])
            nc.sync.dma_start(out=io_t[:], in_=t_io[:, :])
            nc.sync.dma_start(out=di_t[:], in_=t_di[:, :])
            # merge setup DMA deps onto the DVE engine clock
            touch = sb.tile([128, 8], f32)
            nc.vector.tensor_copy(out=touch[:, 0:1], in_=rl_t[:, 0:1])
            nc.vector.tensor_copy(out=touch[:, 1:2], in_=vl_t[:, 0:1])
            nc.vector.tensor_copy(out=touch[:, 2:3], in_=io_t[:, 0:1])
            nc.vector.tensor_copy(out=touch[:, 3:4], in_=idx_t[:, 0:1])
            nc.vector.tensor_copy(out=touch[:, 4:5], in_=di_t[:, 0:1])

            bounce = drp.tile([RPC, 128], fp16)
            tables = [
                drp.tile([SLOTS, 128], fp16, addr_space="Shared",
                         name=f"tbl{k}")
                for k in range(NSTEPS)
            ]
            acc_d = drp.tile([RPC, DIM], f32)

            # acc := e0 slab (fp16 -> fp32 cast during DMA, SWDGE)
            nc.gpsimd.dma_start(
                out=acc_t[:],
                in_=bass.AP(t_emb, 0, [[128, 128], [WR * 128, WPC], [1, DIM]]),
            )
            # initial all-gather of e0 slabs into the shared table
            nc.sync.dma_start(out=bounce[:, :], in_=t_emb[:, :])
            nc.gpsimd.collective_compute(
                "AllGather", mybir.AluOpType.bypass,
                replica_groups=[list(range(NCORES))],
                ins=[bounce[:, :].opt()], outs=[tables[0][:, :].opt()],
            )

            gsz = [GSIZE, GSIZE, GSIZE, SLOTS - 3 * GSIZE]

            for step in range(NSTEPS):
                def body(iv, step=step):
                    xg_t = xgp.tile([128, C_TOT, 128], fp16, name="xg")
                    for g in range(NGRP):
                        nt = int(Cg[g]) * 128
                        nc.gpsimd.dma_gather(
                            out_ap=xg_t[:, int(off_g[g]):int(off_g[g] + Cg[g]), :],
                            in_ap=tables[step][int(g * GSIZE):int(g * GSIZE + gsz[g]), :],
                            idxs_ap=idx_t[:, bass.ds(iv * SW + int(off_g[g] * 8),
                                                     int(Cg[g]) * 8)],
                            num_idxs=nt, num_idxs_reg=nt,
                            elem_size=128, elem_step=128,
                            single_packet=False, queue_num=g,
                        )
                    ps_t = psp.tile([128, DIM], f32, space="PSUM", name="ps")
                    for cj in range(C_TOT):
                        oh_t = ohp.tile([128, 128], fp16, name="oh")
                        nc.vector.tensor_scalar(
                            out=oh_t[:], in0=io_t[:],
                            scalar1=rl_t[:, bass.ds(iv * C_TOT + cj, 1)],
                            scalar2=vl_t[:, bass.ds(iv * C_TOT + cj, 1)],
                            op0=mybir.AluOpType.is_equal,
                            op1=mybir.AluOpType.mult,
                        )
                        nc.tensor.matmul(
                            ps_t[:], lhsT=oh_t[:], rhs=xg_t[:, cj, 0:DIM],
                            start=(cj == 0), stop=(cj == C_TOT - 1),
                        )
                    # acc += d_step
                    nc.vector.tensor_tensor(
                        out=acc_t[:, bass.ds(iv * DIM, DIM)],
                        in0=acc_t[:, bass.ds(iv * DIM, DIM)],
                        in1=ps_t[:], op=mybir.AluOpType.add,
                    )
                    if step < NSTEPS - 1:
                        os_t = osp.tile([128, 128], fp16, name="os")
                        nc.scalar.activation(
                            out=os_t[:, 0:DIM], in_=ps_t[:],
                            func=mybir.ActivationFunctionType.Copy)
                        nc.vector.memset(os_t[:, DIM:128], 0.0)
                        nc.sync.dma_start(
                            out=bounce[bass.ts(iv, WR), :], in_=os_t[:, :])

                with tc.For_i(0, WPC, 1) as iv:
                    body(iv)

                if step < NSTEPS - 1:
                    nc.gpsimd.collective_compute(
                        "AllGather", mybir.AluOpType.bypass,
                        replica_groups=[list(range(NCORES))],
                        ins=[bounce[:, :].opt()],
                        outs=[tables[step + 1][:, :].opt()],
                    )

            # final: gamma for owned drug rows
            nc.sync.dma_start(
                out=bass.AP(acc_d.tensor, 0,
                            [[DIM, 128], [WR * DIM, WPC], [1, DIM]]),
                in_=acc_t[:],
            )
            dg_t = sb.tile([128, DPAD // 128, DIM], f32)
            nc.gpsimd.dma_gather(
                out_ap=dg_t[:, :, :], in_ap=acc_d[:, :], idxs_ap=di_t[:, :],
                num_idxs=DPAD, num_idxs_reg=DPAD,
                elem_size=DIM, elem_step=DIM, single_packet=False,
            )
            sq_t = sb.tile([128, DPAD // 128, DIM], f32)
            nc.vector.tensor_tensor(
                out=sq_t[:, :, :], in0=dg_t[:, :, :], in1=dg_t[:, :, :],
                op=mybir.AluOpType.mult)
            gm_t = sb.tile([128, DPAD // 128, 1], f32)
            nc.vector.tensor_reduce(
                out=gm_t[:, :, :], in_=sq_t[:, :, :],
                axis=mybir.AxisListType.X, op=mybir.AluOpType.add)
            nc.sync.dma_start(out=t_out[:, :], in_=gm_t[:, :, 0])

    nc.compile()
    return nc


def kernel(emb, edge_vals, edge_row, edge_col, drugs):
    from concourse.bass_utils import run_bass_kernel_spmd

    in_maps, drug_pos, Cg, C_TOT = _prep(emb, edge_vals, edge_row, edge_col,
                                         drugs)
    nc = _build(Cg, C_TOT)
    res = run_bass_kernel_spmd(nc, in_maps, core_ids=list(range(NCORES)))
    gamma = np.zeros(N_DRUGS, np.float32)
    for c in range(NCORES):
        out = res.results[c]["gamma"]  # [128, DPAD//128]
        pos = drug_pos[c]
        ii = np.arange(len(pos))
        gamma[pos] = out[ii % 128, ii // 128] / 25.0
    return gamma


# revision 4
# speedup vs baseline: 1.5158x; 1.0420x over previous
"""GNN message-passing (4x SpMM + drug-row squared norms) on 8 trn2 NeuronCores.

Design:
- Nodes are permuted into 784 windows of 128 rows (load-balanced by degree);
  core c owns windows [c*98, (c+1)*98) = 12544 row slots.
- d-state table lives in HBM as [100352, 128] fp16 (64 real dims + 64 pad so
  each row is a 256B dma_gather element).
- Per window: 4 dma_gathers (edge cols bucketed into 4 int16-addressable
  32768-slot groups), one-hot built via DVE tensor_scalar(iota == rowlocal)
  * val, PE matmul-accumulates scatter into PSUM [128 rows, 64].
- After each of steps 1..3, slabs are AllGathered into the shared table.
- acc (= e0+d1+d2+d3+d4) kept in SBUF fp32; final: dma_gather of owned drug
  rows from acc in DRAM, square + reduce on DVE.
Host does sharding/permutation prep and final gamma assembly (gamma/25).
"""
import numpy as np

N_NODES = 100000
N_EDGES = 3200000
DIM = 64
N_DRUGS = 8192
NCORES = 8
NW = 784            # total windows
WR = 128            # rows per window
WPC = NW // NCORES  # 98 windows per core
SLOTS = NW * WR     # 100352
RPC = WPC * WR      # 12544 rows per core
NGRP = 4
GSIZE = 32768
DPAD = 1280         # padded drugs per core
NSTEPS = 4


def _prep(emb, edge_vals, edge_row, edge_col, drugs):
    deg = np.bincount(edge_row, minlength=N_NODES)
    order = np.argsort(-deg, kind="stable")
    slot = np.empty(N_NODES, np.int64)
    ar = np.arange(N_NODES)
    slot[order] = (ar % NW) * WR + (ar // NW)

    er = slot[edge_row.astype(np.int64)]
    w = er >> 7
    rloc = (er & 127).astype(np.float32)
    cs = slot[edge_col.astype(np.int64)]
    g = cs >> 15
    gi = (cs & 32767).astype(np.int16)

    key = w * NGRP + g
    eord = np.argsort(key, kind="stable")
    key_s = key[eord]
    cnt = np.bincount(key_s, minlength=NW * NGRP).reshape(NW, NGRP)
    Cg = np.maximum(np.ceil(cnt.max(axis=0) / 128).astype(np.int64), 1)
    C_TOT = int(Cg.sum())
    off_g = np.zeros(NGRP, np.int64)
    off_g[1:] = np.cumsum(Cg)[:-1]
    SW = C_TOT * 8

    seg_start = np.zeros(NW * NGRP, np.int64)
    seg_start[1:] = np.cumsum(cnt.reshape(-1))[:-1]
    rank = np.arange(N_EDGES) - seg_start[key_s]
    ws = key_s // NGRP
    gs = key_s % NGRP

    rowloc_a = np.zeros((128, NW * C_TOT), np.float32)
    vals_a = np.zeros((128, NW * C_TOT), np.float32)
    ccol = ws * C_TOT + off_g[gs] + rank // 128
    cpart = rank % 128
    rowloc_a[cpart, ccol] = rloc[eord]
    vals_a[cpart, ccol] = edge_vals[eord].astype(np.float32)

    idx16 = np.zeros((16, NW * SW), np.int16)
    icol = ws * SW + off_g[gs] * 8 + rank // 16
    ipart = rank % 16
    idx16[ipart, icol] = gi[eord]
    idx_full = np.tile(idx16, (8, 1))

    emb16 = np.zeros((SLOTS, 128), np.float16)
    emb16[slot, :DIM] = emb.astype(np.float16)

    iota = np.broadcast_to(np.arange(128, dtype=np.float16), (128, 128)).copy()

    dslot = slot[drugs.astype(np.int64)]
    dcore = dslot // RPC
    dloc = (dslot % RPC).astype(np.int16)
    drug_idx = np.zeros((NCORES, 16, DPAD // 16), np.int16)
    drug_pos = []  # per core: original positions, in device token order
    for c in range(NCORES):
        pos = np.nonzero(dcore == c)[0]
        assert len(pos) <= DPAD, f"core {c} owns {len(pos)} drugs > {DPAD}"
        drug_pos.append(pos)
        ii = np.arange(len(pos))
        drug_idx[c, ii % 16, ii // 16] = dloc[pos]
    drug_idx_full = np.tile(drug_idx, (1, 8, 1))

    in_maps = []
    for c in range(NCORES):
        in_maps.append({
            "emb_slab": emb16[c * RPC:(c + 1) * RPC],
            "idx16": np.ascontiguousarray(
                idx_full[:, c * WPC * SW:(c + 1) * WPC * SW]),
            "rowloc": np.ascontiguousarray(
                rowloc_a[:, c * WPC * C_TOT:(c + 1) * WPC * C_TOT]),
            "vals": np.ascontiguousarray(
                vals_a[:, c * WPC * C_TOT:(c + 1) * WPC * C_TOT]),
            "iota": iota,
            "drugidx": drug_idx_full[c],
        })
    return in_maps, drug_pos, Cg, C_TOT


def _build(Cg, C_TOT):
    import concourse.bass as bass
    import concourse.mybir as mybir
    import concourse.tile as tile
    import concourse.bacc as bacc

    SW = C_TOT * 8
    off_g = np.zeros(NGRP, np.int64)
    off_g[1:] = np.cumsum(Cg)[:-1]

    nc = bacc.Bacc("TRN2", target_bir_lowering=False, debug=False,
                   num_devices=NCORES, num_swdge_queues=4)
    fp16 = mybir.dt.float16
    f32 = mybir.dt.float32
    i16 = mybir.dt.int16

    t_emb = nc.dram_tensor("emb_slab", [RPC, 128], fp16, kind="ExternalInput")
    t_idx = nc.dram_tensor("idx16", [128, WPC * SW], i16, kind="ExternalInput")
    t_rl = nc.dram_tensor("rowloc", [128, WPC * C_TOT], f32, kind="ExternalInput")
    t_vl = nc.dram_tensor("vals", [128, WPC * C_TOT], f32, kind="ExternalInput")
    t_io = nc.dram_tensor("iota", [128, 128], fp16, kind="ExternalInput")
    t_di = nc.dram_tensor("drugidx", [128, DPAD // 16], i16, kind="ExternalInput")
    t_out = nc.dram_tensor("gamma", [128, DPAD // 128], f32, kind="ExternalOutput")

    with tile.TileContext(nc) as tc:
        with (
            tc.tile_pool(name="sb", bufs=1) as sb,
            tc.tile_pool(name="xgp", bufs=3) as xgp,
            tc.tile_pool(name="ohp", bufs=8) as ohp,
            tc.tile_pool(name="osp", bufs=3) as osp,
            tc.tile_pool(name="psp", bufs=4, space="PSUM") as psp,
            tc.tile_pool(name="drp", bufs=1, space="DRAM") as drp,
        ):
            idx_t = sb.tile([128, WPC * SW], i16)
            rl_t = sb.tile([128, WPC * C_TOT], f32)
            vl_t = sb.tile([128, WPC * C_TOT], f32)
            io_t = sb.tile([128, 128], fp16)
            di_t = sb.tile([128, DPAD // 16], i16)
            acc_t = sb.tile([128, WPC * DIM], f32)

            nc.sync.dma_start(out=idx_t[:], in_=t_idx[:, :])
            nc.sync.dma_start(out=rl_t[:], in_=t_rl[:, :])
            nc.sync.dma_start(out=vl_t[:], in_=t_vl[:, :])
            nc.sync.dma_start(out=io_t[:], in_=t_io[:, :])
            nc.sync.dma_start(out=di_t[:], in_=t_di[:, :])
            # merge setup DMA deps onto the DVE engine clock
            touch = sb.tile([128, 8], f32)
            nc.vector.tensor_copy(out=touch[:, 0:1], in_=rl_t[:, 0:1])
            nc.vector.tensor_copy(out=touch[:, 1:2], in_=vl_t[:, 0:1])
            nc.vector.tensor_copy(out=touch[:, 2:3], in_=io_t[:, 0:1])
            nc.vector.tensor_copy(out=touch[:, 3:4], in_=idx_t[:, 0:1])
            nc.vector.tensor_copy(out=touch[:, 4:5], in_=di_t[:, 0:1])

            bounce = drp.tile([RPC, 128], fp16)
            tables = [
                drp.tile([SLOTS, 128], fp16, addr_space="Shared",
                         name=f"tbl{k}")
                for k in range(NSTEPS)
            ]
            acc_d = drp.tile([RPC, DIM], f32)

            # acc := e0 slab (fp16 -> fp32 cast during DMA, SWDGE)
            nc.gpsimd.dma_start(
                out=acc_t[:],
                in_=bass.AP(t_emb, 0, [[128, 128], [WR * 128, WPC], [1, DIM]]),
            )
            # initial all-gather of e0 slabs into the shared table
            nc.sync.dma_start(out=bounce[:, :], in_=t_emb[:, :])
            nc.gpsimd.collective_compute(
                "AllGather", mybir.AluOpType.bypass,
                replica_groups=[list(range(NCORES))],
                ins=[bounce[:, :].opt()], outs=[tables[0][:, :].opt()],
            )

            gsz = [GSIZE, GSIZE, GSIZE, SLOTS - 3 * GSIZE]

            for step in range(NSTEPS):
                def body(iv, step=step):
                    xg_t = xgp.tile([128, C_TOT, 128], fp16, name="xg")
                    for g in range(NGRP):
                        nt = int(Cg[g]) * 128
                        nc.gpsimd.dma_gather(
                            out_ap=xg_t[:, int(off_g[g]):int(off_g[g] + Cg[g]), :],
                            in_ap=tables[step][int(g * GSIZE):int(g * GSIZE + gsz[g]), :],
                            idxs_ap=idx_t[:, bass.ds(iv * SW + int(off_g[g] * 8),
                                                     int(Cg[g]) * 8)],
                            num_idxs=nt, num_idxs_reg=nt,
                            elem_size=128, elem_step=128,
                            single_packet=False, queue_num=g,
                        )
                    ps_t = psp.tile([128, DIM], f32, space="PSUM", name="ps")
                    for cj in range(C_TOT):
                        oh_t = ohp.tile([128, 128], fp16, name="oh")
                        nc.vector.tensor_scalar(
                            out=oh_t[:], in0=io_t[:],
                            scalar1=rl_t[:, bass.ds(iv * C_TOT + cj, 1)],
                            scalar2=vl_t[:, bass.ds(iv * C_TOT + cj, 1)],
                            op0=mybir.AluOpType.is_equal,
                            op1=mybir.AluOpType.mult,
                        )
                        nc.tensor.matmul(
                            ps_t[:], lhsT=oh_t[:], rhs=xg_t[:, cj, 0:DIM],
                            start=(cj == 0), stop=(cj == C_TOT - 1),
                        )
                    # acc += d_step
                    nc.vector.tensor_tensor(
                        out=acc_t[:, bass.ds(iv * DIM, DIM)],
                        in0=acc_t[:, bass.ds(iv * DIM, DIM)],
                        in1=ps_t[:], op=mybir.AluOpType.add,
                    )
                    if step < NSTEPS - 1:
                        os_t = osp.tile([128, 128], fp16, name="os")
                        nc.scalar.activation(
                            out=os_t[:, 0:DIM], in_=ps_t[:],
                            func=mybir.ActivationFunctionType.Copy)
                        nc.vector.memset(os_t[:, DIM:128], 0.0)
                        nc.sync.dma_start(
                            out=bounce[bass.ts(iv, WR), :], in_=os_t[:, :])

                tc.For_i_unrolled(0, WPC, 1, body, max_unroll=2)

                if step < NSTEPS - 1:
                    nc.gpsimd.collective_compute(
                        "AllGather", mybir.AluOpType.bypass,
                        replica_groups=[list(range(NCORES))],
                        ins=[bounce[:, :].opt()],
                        outs=[tables[step + 1][:, :].opt()],
                    )

            # final: gamma for owned drug rows
            nc.sync.dma_start(
                out=bass.AP(acc_d.tensor, 0,
                            [[DIM, 128], [WR * DIM, WPC], [1, DIM]]),
                in_=acc_t[:],
            )
            dg_t = sb.tile([128, DPAD // 128, DIM], f32)
            nc.gpsimd.dma_gather(
                out_ap=dg_t[:, :, :], in_ap=acc_d[:, :], idxs_ap=di_t[:, :],
                num_idxs=DPAD, num_idxs_reg=DPAD,
                elem_size=DIM, elem_step=DIM, single_packet=False,
            )
            sq_t = sb.tile([128, DPAD // 128, DIM], f32)
            nc.vector.tensor_tensor(
                out=sq_t[:, :, :], in0=dg_t[:, :, :], in1=dg_t[:, :, :],
                op=mybir.AluOpType.mult)
            gm_t = sb.tile([128, DPAD // 128, 1], f32)
            nc.vector.tensor_reduce(
                out=gm_t[:, :, :], in_=sq_t[:, :, :],
                axis=mybir.AxisListType.X, op=mybir.AluOpType.add)
            nc.sync.dma_start(out=t_out[:, :], in_=gm_t[:, :, 0])

    nc.compile()
    return nc


def kernel(emb, edge_vals, edge_row, edge_col, drugs):
    from concourse.bass_utils import run_bass_kernel_spmd

    in_maps, drug_pos, Cg, C_TOT = _prep(emb, edge_vals, edge_row, edge_col,
                                         drugs)
    nc = _build(Cg, C_TOT)
    res = run_bass_kernel_spmd(nc, in_maps, core_ids=list(range(NCORES)))
    gamma = np.zeros(N_DRUGS, np.float32)
    for c in range(NCORES):
        out = res.results[c]["gamma"]  # [128, DPAD//128]
        pos = drug_pos[c]
        ii = np.arange(len(pos))
        gamma[pos] = out[ii % 128, ii // 128] / 25.0
    return gamma
